# revision 1
# baseline (speedup 1.0000x reference)
"""Multi-head attention (B=8, N=1024, DIM=768, H=12, D=64) on 8 TRN2 NeuronCores.

Sharding: data-parallel over batch — core b computes batch element b end-to-end
(no collectives). Inside each core everything is computed with the "transposed
scores" formulation so no on-chip transposes are needed:

  xT [C, N]  (host pre-transposed)
  qkT[c_out, n] = w_qkv_chunk.T @ xT          (PE, accumulate over C chunks)
  v  [n, c]     = xT_chunk.T @ w_qkv_v        (natural layout, + ones column)
  scoresT[m, n] = kT.T-slice @ qT             (K=64, per head-half, 1-bank psum)
  expT          = exp(SCALE * scoresT)        (ACT, psum->sbuf, bf16 out)
  po[0:64, n]   = [v_h | 1].T @ expT          (accum over m chunks)
  po[64, n]     = colsum (softmax denominator, via the ones column)
  recip         = 1/colsum                    (DVE, via an sbuf staging copy)
  bc            = recip broadcast over partitions (DRAM stride-0 bounce for
                  overlapped pairs; K=1 matmul on the PE for the last pair)
  outT          = po[0:64] * bc               (DVE / Pool)
  out[n, c_out] = outT_chunk.T @ w_proj + bias

Key scheduling facts (measured via TimelineSim + HW bisection):
- every "sc"-tag psum tile is <= 1 bank so the shared pool gives a 4-deep
  rotation: prefetch chains / proj partials slot in without stalling the
  scores double-buffer (psum budget: 4 sc banks + 4 po banks = all 8).
- all input DMAs ride the SP queue as few big multi-dim-AP transfers
  (~500ns descriptor floor per DMA; scalar-queue dma_starts would block the
  ACT sequencer, gpsimd ones take the ~1us/desc software-DGE path).
- the proj contraction is split: k=0..3(+4) accumulated into bf16 accpp
  during pairs 4/5, so after the last pair's normalize only k=4,5, one 4x
  DVE add and the out DMA remain, pipelined per 128-row tile.
- hardware-vs-CoreSim traps (bisected on HW): gpsimd partition_broadcast
  NaNs, reciprocal_approx_fast reading PSUM returns garbage — both avoided.

Dtypes: matmul inputs bf16 by default (1 elem/cycle on PE vs 2 cycles for
f32r); psum accumulation always fp32; output staged bf16 (KERN_OUT_BF16=0
for f32). Env flags KERN_{QKV,SCORES,PROJ}_BF16 select f32r per stage.
"""

import os
import sys

for _p in ("/opt/trn_rl_repo",):
    if os.path.isdir(_p) and _p not in sys.path:
        sys.path.insert(0, _p)

import numpy as np

import concourse.bass as bass
import concourse.mybir as mybir
import concourse.tile as tile
from concourse import bacc

B, N, DIM, H, D = 8, 1024, 768, 12, 64
SCALE = D ** -0.5
F32 = mybir.dt.float32
F32R = mybir.dt.float32r
BF16 = mybir.dt.bfloat16
KC = DIM // 128          # 6 contraction chunks of 128
NT = N // 128            # 8 tiles of 128 along sequence
PAIRS = H // 2           # head pairs per 128-partition tile
VW = D + 1               # v width incl. the ones column

QKV_BF16 = os.environ.get("KERN_QKV_BF16", "1") == "1"
SCORES_BF16 = os.environ.get("KERN_SCORES_BF16", "1") == "1"
PROJ_BF16 = os.environ.get("KERN_PROJ_BF16", "1") == "1"
# gpsimd partition_broadcast works in CoreSim but NaNs on hardware
POOL_BCAST = os.environ.get("KERN_POOL_BCAST", "0") == "1"
PE_BCAST_ALL = os.environ.get("KERN_PE_BCAST_ALL", "0") == "1"
POOL_MUL = os.environ.get("KERN_POOL_MUL", "1") == "1"
OUT_BF16 = os.environ.get("KERN_OUT_BF16", "1") == "1"
OUT_DT = BF16 if OUT_BF16 else F32
MEMSET_ONES = os.environ.get("KERN_MEMSET_ONES", "1") == "1"
MERGED_DMA = os.environ.get("KERN_MERGED_DMA", "1") == "1"
ACT_COPY = os.environ.get("KERN_ACT_COPY", "1") == "1"
# reciprocal_approx_fast (custom DVE op) reading PSUM works in CoreSim but
# produces garbage on hardware — stage the denominator row through SBUF
RECIP_PSUM = os.environ.get("KERN_RECIP_PSUM", "0") == "1"

QKV_DT = BF16 if QKV_BF16 else F32R
QK_DT = BF16 if SCORES_BF16 else F32R
PROJ_DT = BF16 if PROJ_BF16 else F32R


def build_nc():
    nc = bacc.Bacc(None, target_bir_lowering=False, debug=False)
    xT_d = nc.declare_dram_parameter("xT", [DIM, N], QKV_DT, isOutput=False)
    wqkv_d = nc.declare_dram_parameter("w_qkv", [DIM, 3 * DIM], QKV_DT, isOutput=False)
    wproj_d = nc.declare_dram_parameter("w_proj", [DIM, DIM], PROJ_DT, isOutput=False)
    bias_d = nc.declare_dram_parameter("b_proj", [DIM], F32, isOutput=False)
    out_d = nc.declare_dram_parameter("out", [N, DIM], OUT_DT, isOutput=True)

    EXP = mybir.ActivationFunctionType.Exp

    with tile.TileContext(nc) as tc:
        with (
            tc.tile_pool(name="sb", bufs=1) as sb,
            tc.tile_pool(name="ps", bufs=2, space="PSUM") as ps,
            tc.tile_pool(name="dram", bufs=1, space="DRAM") as dpool,
        ):
            # ---- persistent sbuf tiles ----
            # x / w / wproj live in k-major merged tiles so one DMA with a
            # 3D access pattern covers all six 128-row contraction chunks
            # (each DMA pays a ~500ns descriptor floor on the serialized
            # HWDGE pipe — fewer, bigger transfers win)
            w2 = sb.tile([128, KC * 3 * DIM], QKV_DT, tag="w2", name="w2")
            x2 = sb.tile([128, KC * N], QKV_DT, tag="x2", name="x2")
            w_sb = [w2[:, k * 3 * DIM:(k + 1) * 3 * DIM] for k in range(KC)]
            xT_sb = [x2[:, k * N:(k + 1) * N] for k in range(KC)]
            qkT = [sb.tile([128, N], QK_DT, tag=f"qkT{i}", name=f"qkT{i}") for i in range(2 * PAIRS)]
            vaug = [sb.tile([128, H * VW], BF16, tag=f"vaug{i}", name=f"vaug{i}") for i in range(NT)]
            wp2 = sb.tile([128, KC * DIM], PROJ_DT, tag="wp2", name="wp2")
            wproj_sb = [wp2[:, k * DIM:(k + 1) * DIM] for k in range(KC)]
            bias_sb = sb.tile([128, DIM], F32, tag="bias", name="bias")
            outTu = [sb.tile([128, N], PROJ_DT, tag=f"ou{t}", name=f"outTu{t}") for t in range(PAIRS)]
            # bf16 so the tail adds run in the DVE 4x all-sbuf mode
            accpp = [sb.tile([128, DIM], BF16, tag=f"app{i}", name=f"accpp{i}") for i in range(NT)]
            if not POOL_BCAST:
                recip_d = dpool.tile([H, N], F32, name="recip_d")

            # ---- input DMAs ----
            # w_qkv comes in host-permuted column order [q0,k0,q1,k1,...,v].
            # x chunks are split into 512-col halves so the first q0/k0 chains
            # start after only the first halves land. Priority order on the
            # (serialized) DMA pipe: x first halves + q0k0q1k1 columns, then
            # x second halves interleaved with the v columns (vaug runs in
            # pair 0's first phase), then the q2..k5 columns, then proj/bias.
            # All input DMAs go through the SP (sync) queue: a dma_start on
            # the scalar queue head-of-line blocks the ACT sequencer (first
            # exp waits for the whole DGE to drain), and gpsimd dma_start
            # takes the ~1us/descriptor software-DGE path. The single HWDGE
            # pipe serializes transfers anyway, so order = priority:
            # q0/k0 weight cols + x first halves gate the first exp, the v
            # cols gate vaug inside pair 0's first phase, x second halves
            # gate the mt>=4 scores, everything else is prefetch for later.
            w2v = w2[:].rearrange("p (k c) -> p k c", k=KC)
            x2v = x2[:].rearrange("p (k n) -> p k n", k=KC)
            wqv = wqkv_d.rearrange("(k p) c -> p k c", p=128)
            xdv = xT_d.rearrange("(k p) n -> p k n", p=128)

            if MERGED_DMA:
                def dma_w(ks, c0, c1):
                    nc.sync.dma_start(w2v[:, ks, c0:c1], wqv[:, ks, c0:c1])

                def dma_x(ks, n0, n1):
                    nc.sync.dma_start(x2v[:, ks, n0:n1], xdv[:, ks, n0:n1])
            else:
                def dma_w(ks, c0, c1):
                    for k in range(KC)[ks]:
                        nc.sync.dma_start(
                            w_sb[k][:, c0:c1], wqkv_d[k * 128:(k + 1) * 128, c0:c1]
                        )

                def dma_x(ks, n0, n1):
                    for k in range(KC)[ks]:
                        nc.sync.dma_start(
                            xT_sb[k][:, n0:n1], xT_d[k * 128:(k + 1) * 128, n0:n1]
                        )

            dma_w(slice(0, 2), 0, 256)             # q0,k0 columns
            dma_x(slice(0, 2), 0, 512)             # x first halves
            dma_w(slice(2, 4), 0, 256)
            dma_x(slice(2, 4), 0, 512)
            dma_w(slice(4, KC), 0, 256)
            dma_x(slice(4, KC), 0, 512)
            dma_w(slice(0, 3), 1536, 2304)         # v columns (vaug)
            dma_w(slice(3, KC), 1536, 2304)
            dma_x(slice(0, 3), 512, 1024)          # x second halves
            dma_x(slice(3, KC), 512, 1024)
            dma_w(slice(0, KC), 256, 512)          # q1,k1 columns
            dma_w(slice(0, KC), 512, 1536)         # q2..k5 columns
            if MERGED_DMA:
                nc.sync.dma_start(
                    wp2[:].rearrange("p (k c) -> p k c", k=KC),
                    wproj_d.rearrange("(k p) c -> p k c", p=128),
                )
            else:
                for k in range(KC):
                    nc.sync.dma_start(wproj_sb[k], wproj_d[k * 128:(k + 1) * 128, :])
            nc.sync.dma_start(bias_sb[:], bias_d[None, :].to_broadcast((128, DIM)))
            ones16 = sb.tile([128, 64], F32, tag="ones16", name="ones16")
            nc.vector.memset(ones16[:], 1.0)
            # warm the ACT exp table while DMAs run
            warm = sb.tile([1, 8], F32, tag="warm", name="warm")
            nc.vector.memset(warm[:], 0.0)
            nc.scalar.activation(warm[:], warm[:], EXP)

            # All "sc"-tag psum tiles are <= 1 bank (2KB) so the shared pool
            # provides a 4-deep rotation (4 banks): prefetch chains and proj
            # partials slot in without stalling the scores double-buffer.
            def emit_qkT_chain(co, nch):
                # host permutes w_qkv columns to [q0,k0,q1,k1,...,q5,k5,v]
                blk = 2 * co if co < PAIRS else 2 * (co - PAIRS) + 1
                pq = ps.tile([128, 512], F32, tag="sc", bufs=4, name="pq")
                for k in range(KC):
                    nc.tensor.matmul(
                        pq[:],
                        w_sb[k][:, blk * 128:(blk + 1) * 128],
                        xT_sb[k][:, nch * 512:(nch + 1) * 512],
                        start=(k == 0),
                        stop=(k == KC - 1),
                    )
                nc.vector.tensor_copy(qkT[co][:, nch * 512:(nch + 1) * 512], pq[:])

            def emit_vaug_tile(nt):
                # v in natural [n, c] layout, strided into vaug with a ones col
                pva = ps.tile([128, 512], F32, tag="sc", bufs=4, name="pva")
                pvb = ps.tile([128, 256], F32, tag="sc", bufs=4, name="pvb")
                for k in range(KC):
                    nc.tensor.matmul(
                        pva[:],
                        xT_sb[k][:, nt * 128:(nt + 1) * 128],
                        w_sb[k][:, 1536:2048],
                        start=(k == 0),
                        stop=(k == KC - 1),
                    )
                for k in range(KC):
                    nc.tensor.matmul(
                        pvb[:],
                        xT_sb[k][:, nt * 128:(nt + 1) * 128],
                        w_sb[k][:, 2048:2304],
                        start=(k == 0),
                        stop=(k == KC - 1),
                    )
                vv = vaug[nt][:].rearrange("p (h c) -> p h c", h=H)
                nc.vector.tensor_copy(
                    vv[:, 0:8, 0:D],
                    pva[:].rearrange("p (h c) -> p h c", c=D),
                )
                nc.vector.tensor_copy(
                    vv[:, 8:12, 0:D],
                    pvb[:].rearrange("p (h c) -> p h c", c=D),
                )
                if MEMSET_ONES:
                    nc.vector.memset(vv[:, :, D:VW], 1.0)
                else:
                    nc.vector.tensor_copy(vv[:, :, D:VW], ones16[:, 0:H, None])

            def emit_scores(t, mt, nch):
                # per head-half: one matmul into its own 1-bank psum tile
                # and one exp straight after
                exs2 = []
                for half in range(2):
                    rs0, rs1 = 64 * half, 64 * (half + 1)
                    psc = ps.tile([128, 512], F32, tag="sc", bufs=4, name="psc")
                    nc.tensor.matmul(
                        psc[:],
                        qkT[PAIRS + t][rs0:rs1, mt * 128:(mt + 1) * 128],
                        qkT[t][rs0:rs1, nch * 512:(nch + 1) * 512],
                        start=True,
                        stop=True,
                    )
                    ex = sb.tile([128, 512], BF16, tag="ex", bufs=20, name="ex")
                    nc.scalar.activation(ex[:], psc[:], EXP, scale=SCALE)
                    exs2.append(ex)
                return exs2

            def emit_av(t, po, ex2, mt, nch):
                for half in range(2):
                    h = 2 * t + half
                    nc.tensor.matmul(
                        po[half][nch][:],
                        vaug[mt][:, h * VW:(h + 1) * VW],
                        ex2[half][:],
                        start=(mt == 0),
                        stop=(mt == NT - 1),
                    )

            def make_po():
                return [
                    [ps.tile([VW, 512], F32, tag="acc", bufs=4, name=f"po{half}{nch}") for nch in range(2)]
                    for half in range(2)
                ]

            def emit_epilogue(t, po, rec, nch, copies_on_act=False):
                # per (half, nch): copy the attention output rows to sbuf and
                # take the reciprocal of the denominator row straight from
                # psum. For the last pair the copies go to the (by then idle)
                # ACT engine so they run concurrently with the DVE recips.
                for half in range(2):
                    rs0, rs1 = 64 * half, 64 * (half + 1)
                    ncol = slice(nch * 512, (nch + 1) * 512)
                    if RECIP_PSUM:
                        nc.vector.reciprocal_approx_fast(
                            rec[half][0:1, ncol], po[half][nch][D:VW, :]
                        )
                    else:
                        cs = sb.tile([1, 512], F32, tag="cs", bufs=4, name="cs")
                        nc.vector.tensor_copy(cs[:], po[half][nch][D:VW, :])
                        nc.vector.reciprocal_approx_fast(rec[half][0:1, ncol], cs[:])
                    if copies_on_act and ACT_COPY:
                        nc.scalar.copy(outTu[t][rs0:rs1, ncol], po[half][nch][0:D, :])
                    else:
                        nc.vector.tensor_copy(outTu[t][rs0:rs1, ncol], po[half][nch][0:D, :])

            def make_rec():
                return [
                    sb.tile([1, N], F32, tag=f"rec{half}", bufs=2, name=f"rec{half}")
                    for half in range(2)
                ]

            def emit_norm(t, rec, last=False):
                # broadcast each head's recip row over its 64 partitions and
                # scale outTu in place; on the otherwise-idle Pool engine for
                # the overlapped pairs, on DVE for the latency-critical last
                # pair (DVE is free by then and 2x faster than Pool)
                bc = sb.tile([128, N], F32, tag="bc", bufs=2, name="bc")
                if POOL_BCAST:
                    nc.gpsimd.partition_broadcast(bc[0:64, :], rec[0][0:1, :])
                    nc.gpsimd.partition_broadcast(bc[64:128, :], rec[1][0:1, :])
                else:
                    for half in range(2):
                        h = 2 * t + half
                        nc.sync.dma_start(recip_d[h:h + 1, :], rec[half][0:1, :])
                        nc.sync.dma_start(
                            bc[64 * half:64 * half + 64, :],
                            recip_d[h:h + 1, :].to_broadcast((64, N)),
                        )
                if POOL_MUL and not last:
                    nc.gpsimd.tensor_mul(outTu[t][:], outTu[t][:], bc[:])
                else:
                    nc.vector.tensor_mul(outTu[t][:], outTu[t][:], bc[:])

            def emit_norm_last_col(t, rec, nch):
                # one n-column-half (proj tails for nt 0-3 only need the
                # first 512 columns normalized), with the partition
                # broadcast done as a K=1 matmul on the warm-and-idle PE
                ncol = slice(nch * 512, (nch + 1) * 512)
                pbc = ps.tile([128, 512], F32, tag="sc", bufs=4, name="pbc")
                for half in range(2):
                    nc.tensor.matmul(
                        pbc[64 * half:64 * half + 64, :],
                        ones16[0:1, 0:64],
                        rec[half][0:1, ncol],
                        start=True,
                        stop=True,
                    )
                nc.vector.tensor_mul(outTu[t][:, ncol], outTu[t][:, ncol], pbc[:])

            def _proj_mms(nt, ks, pre=None, stop=True):
                if pre is not None:
                    ppa, ppb = pre
                else:
                    ppa = ps.tile([128, 512], F32, tag="sc", bufs=4, name="ppa")
                    ppb = ps.tile([128, 256], F32, tag="sc", bufs=4, name="ppb")
                for i, k in enumerate(ks):
                    nc.tensor.matmul(
                        ppa[:],
                        outTu[k][:, nt * 128:(nt + 1) * 128],
                        wproj_sb[k][:, 0:512],
                        start=(i == 0 and pre is None),
                        stop=(stop and i == len(ks) - 1),
                    )
                for i, k in enumerate(ks):
                    nc.tensor.matmul(
                        ppb[:],
                        outTu[k][:, nt * 128:(nt + 1) * 128],
                        wproj_sb[k][:, 512:768],
                        start=(i == 0 and pre is None),
                        stop=(stop and i == len(ks) - 1),
                    )
                return ppa, ppb

            def emit_proj_partial(nt, ks):
                # accumulate the proj contraction chunks that are already
                # normalized into sbuf, so only k=4,5 remain after pair 5
                ppa, ppb = _proj_mms(nt, ks)
                nc.vector.tensor_add(accpp[nt][:, 0:512], ppa[:], bias_sb[:, 0:512])
                nc.vector.tensor_add(accpp[nt][:, 512:768], ppb[:], bias_sb[:, 512:768])

            def emit_proj_tail(nt, ks, pre=None):
                # stage the last psum chunks to bf16 sbuf on the (idle) ACT
                # engine, then add on DVE in 4x all-sbuf mode
                ppa, ppb = _proj_mms(nt, ks, pre=pre)
                tmp = sb.tile([128, DIM], BF16, tag="ptmp", bufs=3, name="ptmp")
                # split the staging copies so neither ACT nor DVE paces the
                # tail pipeline alone
                nc.scalar.copy(tmp[:, 0:512], ppa[:])
                nc.vector.tensor_copy(tmp[:, 512:768], ppb[:])
                ot = sb.tile([128, DIM], OUT_DT, tag=f"vaug{nt}", name=f"ot{nt}")
                nc.vector.tensor_add(ot[:], accpp[nt][:], tmp[:])
                nc.sync.dma_start(out_d[nt * 128:(nt + 1) * 128, :], ot[:])

            # ---------------- interleaved emission ----------------
            # Startup: only the first n-half chains of q0/k0 gate pair 0.
            emit_qkT_chain(0, 0)
            emit_qkT_chain(PAIRS, 0)

            # ---- pair 0: one n-half at a time; vaug sprinkled in phase A ----
            po0 = make_po()
            rec0 = make_rec()
            LAG0 = 4
            # phase A prefetches: k0 second chain is needed from mt=4 on
            # slot placements tunable for schedule sweeps
            def _slots(env, default):
                v = os.environ.get(env)
                return [int(c) for c in v.split(",")] if v else default

            SA = _slots("KERN_SA", [1, 3])
            SB = _slots("KERN_SB", [1, 3, 5, 7])
            SC = _slots("KERN_SC", [1, 4, 5, 7])
            S4 = _slots("KERN_S4", [5, 6, 7])
            S5 = _slots("KERN_S5", [2, 4, 5, 6, 6])
            pfA = {}
            for s, co in zip(SA, [(PAIRS, 1), (0, 1)]):
                pfA.setdefault(s, []).append(co)
            pfB = {}
            for s, co in zip(SB, [(1, 0), (PAIRS + 1, 0), (1, 1), (PAIRS + 1, 1)]):
                pfB.setdefault(s, []).append(co)
            hoisted = None
            for nch in range(2):
                exs = [None] * NT
                for mt in range(NT):
                    exs[mt] = emit_scores(0, mt, nch)
                    if mt >= LAG0:
                        emit_av(0, po0, exs[mt - LAG0], mt - LAG0, nch)
                    for pf in (pfA if nch == 0 else pfB).get(mt, []):
                        emit_qkT_chain(*pf)
                    if nch == 0:
                        emit_vaug_tile(mt)
                if nch == 1:
                    # keep the exp stream fed through the flush: the next
                    # pair's first scores go in before the tail AVs
                    hoisted = [emit_scores(1, 0, n) for n in range(2)]
                for mt in range(NT - LAG0, NT):
                    emit_av(0, po0, exs[mt], mt, nch)
                emit_epilogue(0, po0, rec0, nch)
            emit_norm(0, rec0)

            # ---- pairs 1..5 ----
            # prefetch pair t+1's chains inside pair t; proj partials for the
            # first contraction chunks run inside pairs 4 and 5
            for t in range(1, PAIRS):
                po = make_po()
                rec = make_rec()
                LAG = 2 if t < PAIRS - 1 else 1
                pf_sched = {mt: [] for mt in range(NT)}
                if t < PAIRS - 1:
                    # odd slots: an "sc"-tag tile emitted between two psc
                    # tiles at mt=0 perturbs the scores double-buffer; pair 4
                    # runs its chains early, ahead of its proj partials
                    SCt = _slots("KERN_SC4", [1, 2, 4, 5]) if t == 4 else SC
                    pf_sched[SCt[0]].append(("chain", (t + 1, 0)))
                    pf_sched[SCt[1]].append(("chain", (PAIRS + t + 1, 0)))
                    pf_sched[SCt[2]].append(("chain", (t + 1, 1)))
                    pf_sched[SCt[3]].append(("chain", (PAIRS + t + 1, 1)))
                if t == 4:
                    # partials read outTu[3], whose normalize lands a few us
                    # into this pair — keep them off the first couple mts
                    pf_sched[S4[0]].append(("proj", (0, range(4))))
                    pf_sched[S4[1]].append(("proj", (1, range(4))))
                    pf_sched[S4[2]].append(("proj", (2, range(4))))
                if t == 5:
                    # nt5-7 include k=4 (norm-4 has completed by mid-pair);
                    # nt3-4 are emitted early so they stop at k=3
                    pf_sched[S5[0]].append(("proj", (3, range(4))))
                    pf_sched[S5[1]].append(("proj", (4, range(4))))
                    pf_sched[S5[2]].append(("proj", (5, range(5))))
                    pf_sched[S5[3]].append(("proj", (6, range(5))))
                    pf_sched[S5[4]].append(("proj", (7, range(5))))
                exs = [None] * NT
                for mt in range(NT):
                    # scores first — the exp stream paces the pair; prefetch
                    # and proj-partial work slots in behind it
                    if mt == 0 and hoisted is not None:
                        exs[0] = hoisted
                    else:
                        pair_ex = []
                        for nch in range(2):
                            pair_ex.append(emit_scores(t, mt, nch))
                        exs[mt] = pair_ex
                    if mt >= LAG:
                        for nch in range(2):
                            emit_av(t, po, exs[mt - LAG][nch], mt - LAG, nch)
                    for kind, args in pf_sched[mt]:
                        if kind == "chain":
                            emit_qkT_chain(*args)
                        else:
                            emit_proj_partial(*args)
                hoisted = None
                if t < PAIRS - 1:
                    hoisted = [emit_scores(t + 1, 0, n) for n in range(2)]
                for mt in range(NT - LAG, NT):
                    for nch in range(2):
                        emit_av(t, po, exs[mt][nch], mt, nch)
                last = t == PAIRS - 1
                if last:
                    # keep PE warm through the norm chain: start the k=4
                    # accumulation for the first output tile now (only one —
                    # the broadcast matmuls need free "sc" psum slots), and
                    # normalize column-half by column-half so the first proj
                    # tails start as early as possible
                    pre = {0: _proj_mms(0, [4], stop=False)}
                    for nch in range(2):
                        emit_epilogue(t, po, rec, nch, copies_on_act=True)
                        emit_norm_last_col(t, rec, nch)
                elif PE_BCAST_ALL:
                    for nch in range(2):
                        emit_epilogue(t, po, rec, nch)
                        emit_norm_last_col(t, rec, nch)
                else:
                    for nch in range(2):
                        emit_epilogue(t, po, rec, nch)
                    emit_norm(t, rec)

            # ---------------- output projection tail ----------------
            for nt in range(NT):
                if nt in pre:
                    emit_proj_tail(nt, [5], pre=pre[nt])
                else:
                    emit_proj_tail(nt, range(4, KC) if nt < 5 else range(5, KC))

    nc.finalize()
    return nc


_NC = None


def _get_nc():
    global _NC
    if _NC is None:
        _NC = build_nc()
    return _NC


def _in_maps(x, w_qkv, w_proj, b_proj):
    import ml_dtypes

    x = np.ascontiguousarray(np.asarray(x, dtype=np.float32))
    w_qkv = np.ascontiguousarray(np.asarray(w_qkv, dtype=np.float32))
    w_proj = np.ascontiguousarray(np.asarray(w_proj, dtype=np.float32))
    b_proj = np.ascontiguousarray(np.asarray(b_proj, dtype=np.float32))
    # permute w_qkv columns to [q0,k0,q1,k1,...,q5,k5,v] so pair-0/1
    # columns can be DMA'd first
    blocks = []
    for t in range(PAIRS):
        blocks.append(w_qkv[:, t * 128:(t + 1) * 128])
        blocks.append(w_qkv[:, 768 + t * 128:768 + (t + 1) * 128])
    blocks.append(w_qkv[:, 1536:2304])
    w_qkv = np.ascontiguousarray(np.concatenate(blocks, axis=1))
    if QKV_BF16:
        w_qkv = np.ascontiguousarray(w_qkv.astype(ml_dtypes.bfloat16))
    if PROJ_BF16:
        w_proj = np.ascontiguousarray(w_proj.astype(ml_dtypes.bfloat16))
    maps = []
    for b in range(B):
        xT = np.ascontiguousarray(x[b].T)
        if QKV_BF16:
            xT = np.ascontiguousarray(xT.astype(ml_dtypes.bfloat16))
        maps.append(
            {
                "xT": xT,
                "w_qkv": w_qkv,
                "w_proj": w_proj,
                "b_proj": b_proj,
            }
        )
    return maps


def kernel(x, w_qkv, w_proj, b_proj):
    from concourse.bass_utils import run_bass_kernel_spmd

    maps = _in_maps(x, w_qkv, w_proj, b_proj)
    res = run_bass_kernel_spmd(_get_nc(), maps, list(range(B)))
    out = np.stack([np.asarray(res.results[c]["out"]) for c in range(B)], axis=0)
    return out.astype(np.float32)


if __name__ == "__main__":
    rng = np.random.default_rng(0)
    x = rng.standard_normal((B, N, DIM), dtype=np.float32)
    w_qkv = rng.standard_normal((DIM, 3 * DIM), dtype=np.float32) * DIM ** -0.5
    w_proj = rng.standard_normal((DIM, DIM), dtype=np.float32) * DIM ** -0.5
    b_proj = rng.standard_normal((DIM,), dtype=np.float32) * 0.01
    out = kernel(x, w_qkv, w_proj, b_proj)
    print(out.shape, out.dtype)

